# revision 27
# baseline (speedup 1.0000x reference)
import sys
sys.path.insert(0, '/opt/trn_rl_repo')
import numpy as np
import ml_dtypes

import concourse.bass as bass
import concourse.bacc as bacc_mod
import concourse.mybir as mybir
from concourse.tile import TileContext
from concourse.masks import make_identity
from concourse.bass_utils import run_bass_kernel_spmd

F32 = mybir.dt.float32
F32R = mybir.dt.float32r
BF16 = mybir.dt.bfloat16
EXP = mybir.ActivationFunctionType.Exp
SIG = mybir.ActivationFunctionType.Sigmoid
TANH = mybir.ActivationFunctionType.Tanh
RELU = mybir.ActivationFunctionType.Relu
GELU = mybir.ActivationFunctionType.Gelu
COPY = mybir.ActivationFunctionType.Copy
IDNT = mybir.ActivationFunctionType.Identity
SQRT = mybir.ActivationFunctionType.Sqrt
AXX = mybir.AxisListType.X
ts, ds = bass.ts, bass.ds

B, S, NL, NU, D, SLOTS, RANK, H = 256, 64, 10000, 2000, 256, 512, 50, 4
NC = 8
RPC, RPAD = 1250, 1280
BLK, NBLK = 256, 5
KCH = (NL + 127) // 128        # 79
KPAD = KCH * 128
BPC = B // NC                  # 32
NT = BPC * S                   # 2048
NTC = NT // 128                # 16
G4 = 4 * D                     # 1024
DH = D // H                    # 64
ISD = 1.0 / np.sqrt(D)
ISH = 1.0 / np.sqrt(DH)
CORE_IDS = list(range(NC))


def _bcast_ap(dram_ap, p=128):
    return bass.AP(tensor=dram_ap.tensor, offset=dram_ap.offset,
                   ap=[[0, p]] + [list(x) for x in dram_ap.ap[1:]])


# ---------------------------------------------------------------------------
# Graph hop
# ---------------------------------------------------------------------------

def build_hop(hop1: bool):
    nc = bacc_mod.Bacc(None, target_bir_lowering=False, debug=True)
    lbT = nc.dram_tensor("lbT", [RANK, RPAD], F32R, kind="ExternalInput")
    radj = nc.dram_tensor("radj", [RANK, NL], F32R, kind="ExternalInput")
    xin = nc.dram_tensor("xin", [KCH * 128, D], F32R, kind="ExternalInput")
    gw = nc.dram_tensor("gw", [D, D], F32R, kind="ExternalInput")
    gb = nc.dram_tensor("gb", [1, D], F32, kind="ExternalInput")
    out = nc.dram_tensor("out", [RPAD, D], F32, kind="ExternalOutput")
    if hop1:
        recip_o = nc.dram_tensor("recip_o", [RPAD, 1], F32, kind="ExternalOutput")
    else:
        recip_i = nc.dram_tensor("recip_i", [RPAD, 1], F32, kind="ExternalInput")
        basei = nc.dram_tensor("basei", [RPAD, D], F32, kind="ExternalInput")

    with TileContext(nc) as tc:
        with (tc.tile_pool(name="const", bufs=1) as cst,
              tc.tile_pool(name="sb", bufs=3) as sb,
              tc.tile_pool(name="sb2", bufs=2) as sb2,
              tc.tile_pool(name="eps_p", bufs=2, space="PSUM") as eps_p,
              tc.tile_pool(name="z_p", bufs=1, space="PSUM") as z_p,
              tc.tile_pool(name="tp_p", bufs=2, space="PSUM") as tp_p,
              tc.tile_pool(name="nb_p", bufs=2, space="PSUM") as nb_p):
            lbT_sb = cst.tile([RANK, RPAD], F32R, tag="lbT")
            nc.sync.dma_start(out=lbT_sb, in_=lbT[:, :])
            radj_sb = cst.tile([RANK, NL], F32R, tag="radj")
            nc.sync.dma_start(out=radj_sb, in_=radj[:, :])
            xin_sb = cst.tile([128, KCH * D], F32R, tag="xin")
            nc.sync.dma_start(out=xin_sb,
                              in_=xin.rearrange("(k p) d -> p k d", p=128))
            gw_sb = cst.tile([128, 2, D], F32R, tag="gw")
            nc.sync.dma_start(out=gw_sb[:, 0, :], in_=gw[0:128, :])
            nc.sync.dma_start(out=gw_sb[:, 1, :], in_=gw[128:256, :])
            gbb = cst.tile([128, D], F32, tag="gbb")
            nc.sync.dma_start(out=gbb, in_=_bcast_ap(gb[:, :]))
            ident = cst.tile([128, 128], F32, tag="ident")
            make_identity(nc, ident)

            for m in range(NBLK):
                rows = ts(m, BLK)
                zA = z_p.tile([128, BLK], F32, tag="zA")
                zB = z_p.tile([128, BLK], F32, tag="zB")
                eacc = sb.tile([128, BLK], F32, tag="eacc")
                if hop1:
                    nc.vector.memset(eacc, 0.0)
                for k in range(KCH):
                    pk = min(128, NL - k * 128)
                    eps = eps_p.tile([128, BLK], F32, tag="eps")
                    nc.tensor.matmul(eps[:pk], lhsT=radj_sb[:, k * 128:k * 128 + pk],
                                     rhs=lbT_sb[:, rows], start=True, stop=True)
                    e = sb.tile([128, BLK], F32R, tag="e")
                    nc.scalar.activation(out=e[:pk], in_=eps[:pk], func=EXP)
                    nc.tensor.matmul(zA, lhsT=xin_sb[:pk, k * D:k * D + 128],
                                     rhs=e[:pk], start=(k == 0), stop=(k == KCH - 1))
                    nc.tensor.matmul(zB, lhsT=xin_sb[:pk, k * D + 128:k * D + 256],
                                     rhs=e[:pk], start=(k == 0), stop=(k == KCH - 1))
                    if hop1:
                        nc.vector.tensor_add(eacc[:pk], eacc[:pk], e[:pk])
                zsA = sb2.tile([128, BLK], F32R, tag="zsA")
                nc.scalar.activation(out=zsA, in_=zA, func=COPY)
                zsB = sb2.tile([128, BLK], F32R, tag="zsB")
                nc.scalar.activation(out=zsB, in_=zB, func=COPY)
                for h in range(2):
                    rh = ds(m * BLK + h * 128, 128)
                    recip = sb.tile([128, 1], F32, tag="recip")
                    if hop1:
                        tp = tp_p.tile([128, 128], F32, tag="tp")
                        nc.tensor.transpose(tp, eacc[:, h * 128:(h + 1) * 128], ident)
                        rsum = sb.tile([128, 1], F32, tag="rsum")
                        nc.vector.reduce_sum(out=rsum, in_=tp, axis=AXX)
                        nc.vector.reciprocal(out=recip, in_=rsum)
                        nc.sync.dma_start(out=recip_o[rh, :], in_=recip)
                    else:
                        nc.sync.dma_start(out=recip, in_=recip_i[rh, :])
                    nb = nb_p.tile([128, D], F32, tag="nb")
                    nc.tensor.matmul(nb, lhsT=zsA[:, h * 128:(h + 1) * 128],
                                     rhs=gw_sb[:, 0, :], start=True, stop=False)
                    nc.tensor.matmul(nb, lhsT=zsB[:, h * 128:(h + 1) * 128],
                                     rhs=gw_sb[:, 1, :], start=False, stop=True)
                    nc.vector.tensor_scalar_mul(nb, nb, recip)
                    nc.vector.tensor_add(nb, nb, gbb)
                    ot = sb.tile([128, D], F32, tag="ot")
                    if hop1:
                        nc.scalar.activation(out=ot, in_=nb, func=RELU)
                    else:
                        bt = sb.tile([128, D], F32, tag="bt")
                        nc.sync.dma_start(out=bt, in_=basei[rh, :])
                        nc.vector.tensor_add(ot, nb, bt)
                    nc.sync.dma_start(out=out[rh, :], in_=ot)
    nc.finalize()
    return nc


# ---------------------------------------------------------------------------
# Sequence program
# ---------------------------------------------------------------------------

def build_seq(affine: bool):
    nc = bacc_mod.Bacc(None, target_bir_lowering=False, debug=True)

    def I(name, shape, dt=F32):
        return nc.dram_tensor(name, shape, dt, kind="ExternalInput")

    x_bp = I("x_bp", [NT, D])
    xT = I("xT", [D, S, BPC], BF16)
    wihT0 = I("wihT0", [D, G4], BF16)
    whhT0 = I("whhT0", [D, G4], BF16)
    whhT1 = I("whhT1", [D, G4], BF16)
    wihT1 = I("wihT1", [D, G4], BF16)
    bias0 = I("bias0", [1, G4])
    bias1 = I("bias1", [1, G4])
    ain_wT = I("ain_wT", [D, 2 * D], F32R)
    av_wT = I("av_wT", [D, D], F32R)
    ain_qkb = I("ain_qkb", [2 * D, 1])
    ain_vb = I("ain_vb", [1, D])
    aout_wT = I("aout_wT", [D, D], F32R)
    aout_b = I("aout_b", [1, D])
    lng = {}; lnb = {}
    for j in (1, 2, 3):
        lng[j] = I(f"n{j}g", [1, D]); lnb[j] = I(f"n{j}b", [1, D])
    rq_wT = I("rq_wT", [D, D], F32R); rq_b = I("rq_b", [D, 1])
    rk_wT = I("rk_wT", [D, D], F32R); rk_b = I("rk_b", [D, 1])
    memT = I("memT", [D, SLOTS], F32R)
    mem_aug = I("mem_aug", [SLOTS, D + 2], F32R)
    wg_wT = I("wg_wT", [D, 1], F32R); wg_b = I("wg_b", [1, 1])
    pq_wT = I("pq_wT", [D, D], F32R); pq_b = I("pq_b", [D, 1])
    pk_wT = I("pk_wT", [D, D], F32R); pk_b = I("pk_b", [D, 1])
    pv_wT = I("pv_wT", [D, D], F32R); pv_b = I("pv_b", [1, D])
    op1_wT = I("op1_wT", [3 * D, 2 * D], BF16)
    op1_b = I("op1_b", [2 * D, 1])
    onesc = I("onesc", [128, 1], BF16)
    hid_o = nc.dram_tensor("hid_o", [BPC, 2 * D], F32, kind="ExternalOutput")
    cur_o = nc.dram_tensor("cur_o", [BPC, D], F32, kind="ExternalOutput")
    gate_o = nc.dram_tensor("gate_o", [1, BPC], F32, kind="ExternalOutput")

    with TileContext(nc) as tc:
        with (tc.tile_pool(name="cst", bufs=1) as cst,
              tc.tile_pool(name="bigp", bufs=1) as bigp,
              tc.tile_pool(name="sb", bufs=3) as sb,
              tc.tile_pool(name="ps4", bufs=2, space="PSUM") as ps4,
              tc.tile_pool(name="g_p", bufs=1, space="PSUM") as g_p,
              tc.tile_pool(name="t_p", bufs=2, space="PSUM") as t_p):

            def bc(ap, tag, n=D, p=128, dt=F32):
                t = cst.tile([p, n], dt, tag=tag, name=tag)
                nc.gpsimd.dma_start(out=t, in_=_bcast_ap(ap[:, :], p))
                return t

            def load2(dr, n, tag, dt=F32R, chunks=2):
                t = cst.tile([128, chunks, n], dt, tag=tag, name=tag)
                for c in range(chunks):
                    nc.gpsimd.dma_start(out=t[:, c], in_=dr[c * 128:(c + 1) * 128])
                return t

            identF = cst.tile([128, 128], F32, tag="identF")
            make_identity(nc, identF)
            i32stk = cst.tile([64, 32], BF16, tag="i32stk")
            for j in range(2):
                make_identity(nc, i32stk[32 * j:32 * (j + 1), :])
            i32f = cst.tile([32, 32], F32, tag="i32f")
            make_identity(nc, i32f)
            eps_sb = cst.tile([128, 1], F32, tag="eps_sb")
            nc.vector.memset(eps_sb, 1e-5)

            ones_sb = cst.tile([128, 1], BF16, tag="ones_sb")
            nc.sync.dma_start(out=ones_sb, in_=onesc[:, :])
            xT_sb = bigp.tile([128, 2, S * BPC], BF16, tag="bigE", name="xT_sb")
            for c in range(2):
                nc.sync.dma_start(out=xT_sb[:, c], in_=xT[c * 128:(c + 1) * 128])
            x_bp_sb = bigp.tile([128, NTC, D], F32, tag="bigB", name="x_bp_sb")
            for i in range(NTC):
                nc.sync.dma_start(out=x_bp_sb[:, i, :], in_=x_bp[ts(i, 128), :])
            wih0_sb = load2(wihT0, G4, "wih0_sb", BF16)
            whh0_sb = load2(whhT0, G4, "whh0_sb", BF16)
            whh1_sb = load2(whhT1, G4, "whh1_sb", BF16)
            wih1_sb = load2(wihT1, G4, "wih1_sb", BF16)
            b0_full = bc(bias0, "b0_full", G4)
            bias1_b32 = bc(bias1, "bias1_b32", G4, 32, BF16)

            # ---- xg1 = x @ wihT0 (+bias0), tg-packed, bf16 ----
            xg1 = bigp.tile([64, 16, G4], BF16, tag="bigA", name="xg1")
            xv = xT_sb.rearrange("p c (tsub tg b) -> p c tsub tg b", tsub=16, b=BPC)

            def emit_xg_wave(w):
                for tsub in range(16):
                    for nh in range(2):
                        ps = ps4.tile([64, 512], F32, tag="ps")
                        for kc in range(2):
                            nc.tensor.matmul(
                                ps, lhsT=xv[:, kc, tsub, 2 * w:2 * w + 2, :],
                                rhs=wih0_sb[:, kc, ts(nh, 512)],
                                start=(kc == 0), stop=(kc == 1))
                        if tsub % 2 == 0:
                            nc.scalar.activation(out=xg1[:, tsub, ts(nh, 512)],
                                                 in_=ps, func=COPY)
                        else:
                            nc.vector.tensor_copy(out=xg1[:, tsub, ts(nh, 512)],
                                                  in_=ps)
                    nc.vector.tensor_add(xg1[:, tsub, :], xg1[:, tsub, :],
                                         b0_full[0:64])

            # ---- LSTM ----
            h1T = cst.tile([128, 2, BPC], BF16, tag="h1T")
            h2T_all = bigp.tile([128, 2, BPC, S], BF16, tag="bigC", name="h2T_all")
            c1 = cst.tile([BPC, D], F32, tag="c1")
            c2 = cst.tile([BPC, D], F32, tag="c2")
            nc.vector.memset(h1T, 0.0)
            nc.vector.memset(c1, 0.0)
            nc.vector.memset(c2, 0.0)

            def lstm_step(t, layer):
                glo = g_p.tile([BPC, 512], F32, tag="glo")
                ghi = g_p.tile([BPC, 512], F32, tag="ghi")
                if layer == 1:
                    tg, tsub = t // 16, t % 16
                    tg1 = tg % 2
                    xs = xg1[tg1 * 32:(tg1 + 1) * 32, tsub]
                    ii = i32stk[tg1 * 32:(tg1 + 1) * 32, :]
                    nc.tensor.matmul(glo, lhsT=ii, rhs=xs[:, 0:512],
                                     start=True, stop=False)
                    nc.tensor.matmul(ghi, lhsT=ii, rhs=xs[:, 512:1024],
                                     start=True, stop=False)
                    for kc in range(2):
                        nc.tensor.matmul(glo, lhsT=h1T[:, kc],
                                         rhs=whh0_sb[:, kc, 0:512],
                                         start=False, stop=(kc == 1))
                        nc.tensor.matmul(ghi, lhsT=h1T[:, kc],
                                         rhs=whh0_sb[:, kc, 512:1024],
                                         start=False, stop=(kc == 1))
                else:
                    nc.tensor.matmul(glo, lhsT=i32stk[0:32, :], rhs=bias1_b32[:, 0:512],
                                     start=True, stop=False)
                    nc.tensor.matmul(ghi, lhsT=i32stk[0:32, :], rhs=bias1_b32[:, 512:1024],
                                     start=True, stop=False)
                    for kc in range(2):
                        last = (kc == 1)
                        if t > 0:
                            nc.tensor.matmul(glo, lhsT=h2T_all[:, kc, :, t - 1],
                                             rhs=whh1_sb[:, kc, 0:512],
                                             start=False, stop=False)
                            nc.tensor.matmul(ghi, lhsT=h2T_all[:, kc, :, t - 1],
                                             rhs=whh1_sb[:, kc, 512:1024],
                                             start=False, stop=False)
                        nc.tensor.matmul(glo, lhsT=h1T[:, kc],
                                         rhs=wih1_sb[:, kc, 0:512],
                                         start=False, stop=last)
                        nc.tensor.matmul(ghi, lhsT=h1T[:, kc],
                                         rhs=wih1_sb[:, kc, 512:1024],
                                         start=False, stop=last)
                cprev = c1 if layer == 1 else c2
                sif = sb.tile([BPC, 512], F32, tag="sif")
                nc.scalar.activation(out=sif, in_=glo, func=SIG)
                tg_ = sb.tile([BPC, D], F32, tag="tg_")
                nc.scalar.activation(out=tg_, in_=ghi[:, 0:D], func=TANH)
                so = sb.tile([BPC, D], F32, tag="so")
                nc.scalar.activation(out=so, in_=ghi[:, D:2 * D], func=SIG)
                nc.vector.tensor_mul(cprev, sif[:, 256:512], cprev)
                ig = sb.tile([BPC, D], F32, tag="ig")
                nc.vector.tensor_mul(ig, sif[:, 0:256], tg_)
                nc.vector.tensor_add(cprev, cprev, ig)
                tc_ = sb.tile([BPC, D], F32, tag="tc_")
                nc.scalar.activation(out=tc_, in_=cprev, func=TANH)
                hnew = sb.tile([BPC, D], F32, tag="hnew")
                nc.vector.tensor_mul(hnew, so, tc_)
                for c in range(2):
                    tp = t_p.tile([128, BPC], F32, tag="tp")
                    nc.tensor.transpose(tp, hnew[:, c * 128:(c + 1) * 128],
                                        i32f[:, :])
                    dst = h1T[:, c] if layer == 1 else h2T_all[:, c, :, t]
                    nc.scalar.activation(out=dst, in_=tp, func=COPY)

            emit_xg_wave(0)
            for t in range(S):
                if t == 32:
                    emit_xg_wave(1)
                lstm_step(t, 1)
                lstm_step(t, 2)

            # ---- res1 + LN1 ----
            ng = {j: bc(lng[j][:, :], f"n{j}g_b") for j in (1, 2, 3)} if affine else {}
            nb_ = {j: bc(lnb[j][:, :], f"n{j}b_b") for j in (1, 2, 3)} if affine else {}

            def ln_rows(xi, j, p=128):
                stats = sb.tile([p, 6], F32, tag="lnst", name="lnst")
                nc.vector.bn_stats(out=stats, in_=xi)
                mv = sb.tile([p, 2], F32, tag="lnmv", name="lnmv")
                nc.vector.bn_aggr(out=mv, in_=stats)
                std = sb.tile([p, 1], F32, tag="lnsd", name="lnsd")
                nc.scalar.activation(out=std, in_=mv[:, 1:2], func=SQRT,
                                     bias=eps_sb[:p, :])
                rstd = sb.tile([p, 1], F32, tag="lnrs", name="lnrs")
                nc.vector.reciprocal(out=rstd, in_=std)
                nmu = sb.tile([p, 1], F32, tag="lnnm", name="lnnm")
                nc.vector.tensor_mul(nmu, mv[:, 0:1], rstd)
                nc.vector.tensor_scalar_mul(nmu, nmu, -1.0)
                nc.vector.tensor_scalar_mul(xi, xi, rstd)
                nc.vector.tensor_scalar_add(xi, xi, nmu)
                if affine:
                    nc.vector.tensor_mul(xi, xi, ng[j][:p])
                    nc.vector.tensor_add(xi, xi, nb_[j][:p])

            lout_bp = bigp.tile([128, NTC, D], F32, tag="bigD", name="lout_bp")
            for i in range(NTC):
                for c in range(2):
                    tt = sb.tile([128, 128], BF16, tag="dmat")
                    nc.sync.dma_start_transpose(out=tt,
                                                in_=h2T_all[:, c, 2 * i:2 * i + 2, :])
                    nc.vector.tensor_add(lout_bp[:, i, c * 128:(c + 1) * 128],
                                         tt, x_bp_sb[:, i, c * 128:(c + 1) * 128])
            for i in range(NTC):
                ln_rows(lout_bp[:, i, :], 1)

            # ---- qkT, v_bp ----
            loutT = bigp.tile([128, 2, NT], F32R, tag="bigA", name="loutT")
            for i in range(NTC):
                for c in range(2):
                    tp = t_p.tile([128, 128], F32, tag="tp")
                    nc.tensor.transpose(tp, lout_bp[:, i, c * 128:(c + 1) * 128], identF)
                    nc.scalar.activation(out=loutT[:, c, ts(i, 128)], in_=tp, func=COPY)
            ain_sb = load2(ain_wT, 2 * D, "ain_sb")
            aqkb_sb = load2(ain_qkb, 1, "aqkb_sb", F32, 4)
            qkT = bigp.tile([128, 4, NT], BF16, tag="bigB", name="qkT")
            for mc in range(4):
                for nch in range(4):
                    ps = ps4.tile([128, 512], F32, tag="ps")
                    for kc in range(2):
                        nc.tensor.matmul(ps, lhsT=ain_sb[:, kc, ts(mc, 128)],
                                         rhs=loutT[:, kc, ts(nch, 512)],
                                         start=(kc == 0), stop=(kc == 1))
                    nc.scalar.activation(out=qkT[:, mc, ts(nch, 512)], in_=ps,
                                         func=IDNT, bias=aqkb_sb[:, mc])
            av_sb = load2(av_wT, D, "av_sb")
            avb_b = bc(ain_vb[:, :], "avb_b")
            v_bp = bigp.tile([128, NTC, D], BF16, tag="bigC", name="v_bp")
            for i in range(NTC):
                ps = ps4.tile([128, D], F32, tag="ps")
                for kc in range(2):
                    nc.tensor.matmul(ps, lhsT=loutT[:, kc, ts(i, 128)],
                                     rhs=av_sb[:, kc], start=(kc == 0), stop=(kc == 1))
                nc.vector.tensor_add(v_bp[:, i, :], ps, avb_b)

            # ---- attention ----
            attn_bp = bigp.tile([128, NTC, D], F32, tag="bigE", name="attn_bp")
            for b in range(BPC):
                i, r0 = b // 2, (b % 2) * 64
                vb = v_bp[r0:r0 + 64, i]
                op = t_p.tile([64, 264], F32, tag="op", bufs=1)
                for hh in range(H):
                    qh = qkT[(hh % 2) * 64:(hh % 2) * 64 + 64, hh // 2,
                             b * S:(b + 1) * S]
                    kh = qkT[(hh % 2) * 64:(hh % 2) * 64 + 64, 2 + hh // 2,
                             b * S:(b + 1) * S]
                    sT = t_p.tile([128, 64], F32, tag="tp")
                    nc.tensor.matmul(sT[r0:r0 + 64], lhsT=kh, rhs=qh,
                                     start=True, stop=True)
                    eT = sb.tile([128, 64], BF16, tag="eT")
                    nc.scalar.activation(out=eT[r0:r0 + 64], in_=sT[r0:r0 + 64],
                                         func=EXP, scale=ISH)
                    nc.tensor.matmul(op[:, hh * 64:(hh + 1) * 64],
                                     lhsT=eT[r0:r0 + 64],
                                     rhs=vb[:, hh * 64:(hh + 1) * 64],
                                     start=True, stop=True)
                    nc.tensor.matmul(op[:, 256 + hh:257 + hh],
                                     lhsT=eT[r0:r0 + 64],
                                     rhs=ones_sb[r0:r0 + 64, :],
                                     start=True, stop=True)
                rec4 = sb.tile([64, 4], F32, tag="rec4")
                nc.vector.reciprocal(out=rec4, in_=op[:, 256:260])
                for hh in range(H):
                    nc.vector.tensor_scalar_mul(
                        attn_bp[r0:r0 + 64, i, hh * 64:(hh + 1) * 64],
                        op[:, hh * 64:(hh + 1) * 64], rec4[:, hh:hh + 1])

            # ---- proj + residual + LN2 ----
            attnT = bigp.tile([128, 2, NT], F32R, tag="bigA", name="attnT")
            for i in range(NTC):
                for c in range(2):
                    tp = t_p.tile([128, 128], F32, tag="tp")
                    nc.tensor.transpose(tp, attn_bp[:, i, c * 128:(c + 1) * 128], identF)
                    nc.scalar.activation(out=attnT[:, c, ts(i, 128)], in_=tp, func=COPY)
            aout_sb = load2(aout_wT, D, "aout_sb")
            aout_b_b = bc(aout_b[:, :], "aout_b_b")
            for i in range(NTC):
                ps = ps4.tile([128, D], F32, tag="ps")
                for kc in range(2):
                    nc.tensor.matmul(ps, lhsT=attnT[:, kc, ts(i, 128)],
                                     rhs=aout_sb[:, kc], start=(kc == 0), stop=(kc == 1))
                nc.vector.tensor_add(ps, ps, aout_b_b)
                nc.vector.tensor_add(lout_bp[:, i, :], lout_bp[:, i, :], ps)
            for i in range(NTC):
                ln_rows(lout_bp[:, i, :], 2)
            # lout_bp is now attn_out

            # ---- cur, curT ----
            cur = cst.tile([BPC, D], F32, tag="cur")
            nc.sync.dma_start(out=cur[0::2, :], in_=lout_bp[63:64, :, :])
            nc.sync.dma_start(out=cur[1::2, :], in_=lout_bp[127:128, :, :])
            curT = cst.tile([128, 2, BPC], BF16, tag="curT")
            for c in range(2):
                tp = t_p.tile([128, BPC], F32, tag="tp")
                nc.tensor.transpose(tp, cur[:, c * 128:(c + 1) * 128], i32f[:, :])
                nc.scalar.activation(out=curT[:, c], in_=tp, func=COPY)
            nc.sync.dma_start(out=cur_o[:, :], in_=cur)

            def matvecT(wT_dr, b_dr, tag):
                wsb = load2(wT_dr, D, f"w_{tag}", BF16)
                bsb = load2(b_dr, 1, f"b_{tag}", F32)
                res = cst.tile([128, 2, BPC], BF16, tag=f"r_{tag}", name=f"r_{tag}")
                for mc in range(2):
                    ps = t_p.tile([128, BPC], F32, tag="tp")
                    for kc in range(2):
                        nc.tensor.matmul(ps, lhsT=wsb[:, kc, ts(mc, 128)],
                                         rhs=curT[:, kc], start=(kc == 0), stop=(kc == 1))
                    nc.scalar.activation(out=res[:, mc], in_=ps, func=IDNT,
                                         bias=bsb[:, mc])
                return res

            # ---- memory read + LN3 -> menh ----
            mqT = matvecT(rq_wT, rq_b, "rq")
            rk_sb = load2(rk_wT, D, "rk_sb")  # stays f32r: N=512 fine
            rkb_sb = load2(rk_b, 1, "rkb_sb", F32)
            memT_sb = load2(memT, SLOTS, "memT_sb")
            mkT = cst.tile([128, 2, SLOTS], BF16, tag="mkT")
            for mc in range(2):
                ps = ps4.tile([128, SLOTS], F32, tag="ps")
                for kc in range(2):
                    nc.tensor.matmul(ps, lhsT=rk_sb[:, kc, ts(mc, 128)],
                                     rhs=memT_sb[:, kc], start=(kc == 0), stop=(kc == 1))
                nc.scalar.activation(out=mkT[:, mc], in_=ps, func=IDNT,
                                     bias=rkb_sb[:, mc])
            maug_sb = load2(mem_aug, D + 2, "maug_sb", F32R, 4)
            expT = cst.tile([128, 4, BPC], F32R, tag="expT")
            for sc in range(4):
                ps = t_p.tile([128, BPC], F32, tag="tp")
                for kc in range(2):
                    nc.tensor.matmul(ps, lhsT=mkT[:, kc, ts(sc, 128)],
                                     rhs=mqT[:, kc], start=(kc == 0), stop=(kc == 1))
                nc.scalar.activation(out=expT[:, sc], in_=ps, func=EXP, scale=ISD)
            mrd = t_p.tile([BPC, D + 2], F32, tag="tp")
            for sc in range(4):
                nc.tensor.matmul(mrd, lhsT=expT[:, sc], rhs=maug_sb[:, sc],
                                 start=(sc == 0), stop=(sc == 3))
            mrec = sb.tile([BPC, 1], F32, tag="mrec")
            nc.vector.reciprocal(out=mrec, in_=mrd[:, D:D + 1])
            menh = cst.tile([BPC, D], F32, tag="menh")
            nc.vector.tensor_scalar_mul(menh, mrd[:, 0:D], mrec)
            nc.vector.tensor_add(menh, menh, cur)
            ln_rows(menh, 3, BPC)

            # ---- gate ----
            wg_sb = load2(wg_wT, 1, "wg_sb", BF16)
            wgb_sb = cst.tile([1, 1], F32, tag="wgb_sb")
            nc.sync.dma_start(out=wgb_sb, in_=wg_b[:, :])
            gps = t_p.tile([1, BPC], F32, tag="tp")
            for kc in range(2):
                nc.tensor.matmul(gps, lhsT=wg_sb[:, kc], rhs=curT[:, kc],
                                 start=(kc == 0), stop=(kc == 1))
            gate_sb = sb.tile([1, BPC], F32, tag="gate")
            nc.scalar.activation(out=gate_sb, in_=gps, func=SIG, bias=wgb_sb[:, :])
            nc.sync.dma_start(out=gate_o[:, :], in_=gate_sb)

            # ---- pointer ----
            pqT = matvecT(pq_wT, pq_b, "pq")
            # post-LN2 transpose for pk/pv
            aoT = bigp.tile([128, 2, NT], F32R, tag="bigA", name="aoT")
            for i in range(NTC):
                for c in range(2):
                    tp = t_p.tile([128, 128], F32, tag="tp")
                    nc.tensor.transpose(tp, lout_bp[:, i, c * 128:(c + 1) * 128], identF)
                    nc.scalar.activation(out=aoT[:, c, ts(i, 128)], in_=tp, func=COPY)
            pk_sb = load2(pk_wT, D, "pk_sb")
            pkb_sb = load2(pk_b, 1, "pkb_sb", F32)
            pkT = bigp.tile([128, 2, NT], BF16, tag="bigB", name="pkT")
            for mc in range(2):
                for nch in range(4):
                    ps = ps4.tile([128, 512], F32, tag="ps")
                    for kc in range(2):
                        nc.tensor.matmul(ps, lhsT=pk_sb[:, kc, ts(mc, 128)],
                                         rhs=aoT[:, kc, ts(nch, 512)],
                                         start=(kc == 0), stop=(kc == 1))
                    nc.scalar.activation(out=pkT[:, mc, ts(nch, 512)], in_=ps,
                                         func=IDNT, bias=pkb_sb[:, mc])
            pv_sb = load2(pv_wT, D, "pv_sb")
            pvb_b = bc(pv_b[:, :], "pvb_b")
            pv_bp = bigp.tile([128, NTC, D], BF16, tag="bigC", name="pv_bp")
            for i in range(NTC):
                ps = ps4.tile([128, D], F32, tag="ps")
                for kc in range(2):
                    nc.tensor.matmul(ps, lhsT=aoT[:, kc, ts(i, 128)],
                                     rhs=pv_sb[:, kc], start=(kc == 0), stop=(kc == 1))
                nc.vector.tensor_add(pv_bp[:, i, :], ps, pvb_b)
            wps = t_p.tile([S, BPC], F32, tag="tp")
            for b in range(BPC):
                for kc in range(2):
                    nc.tensor.matmul(wps[:, b:b + 1],
                                     lhsT=pkT[:, kc, b * S:(b + 1) * S],
                                     rhs=pqT[:, kc, b:b + 1],
                                     start=(kc == 0), stop=(kc == 1))
            ewps = sb.tile([128, BPC], BF16, tag="ewps")
            nc.scalar.activation(out=ewps[0:S], in_=wps, func=EXP, scale=ISD)
            nc.sync.dma_start(out=ewps[S:2 * S], in_=ewps[0:S])
            ptrT_ps = t_p.tile([128, 96], F32, tag="p0", bufs=1)
            for b in range(BPC):
                i, r0 = b // 2, (b % 2) * 64
                lh = pv_bp[r0:r0 + 64, i]
                w1 = ewps[r0:r0 + 64, b:b + 1]
                nc.tensor.matmul(ptrT_ps[:, b:b + 1], lhsT=lh[:, 0:128], rhs=w1,
                                 start=True, stop=True)
                nc.tensor.matmul(ptrT_ps[:, 32 + b:33 + b], lhsT=lh[:, 128:256],
                                 rhs=w1, start=True, stop=True)
                nc.tensor.matmul(ptrT_ps[0:1, 64 + b:65 + b],
                                 lhsT=ones_sb[r0:r0 + 64, :], rhs=w1,
                                 start=True, stop=True)
            psum_sb = sb.tile([1, BPC], F32, tag="psmsb")
            nc.scalar.activation(out=psum_sb, in_=ptrT_ps[0:1, 64:96], func=COPY)
            prT = t_p.tile([BPC, 1], F32, tag="tp")
            nc.tensor.transpose(prT, psum_sb, i32f[:1, :1])
            prec = sb.tile([BPC, 1], F32, tag="prec")
            nc.vector.reciprocal(out=prec, in_=prT)
            ptrT_sb = sb.tile([128, 2, BPC], F32, tag="ptrTsb")
            nc.scalar.activation(out=ptrT_sb[:, 0], in_=ptrT_ps[:, 0:32], func=COPY)
            nc.scalar.activation(out=ptrT_sb[:, 1], in_=ptrT_ps[:, 32:64], func=COPY)
            ptr = cst.tile([BPC, D], F32, tag="ptr")
            for c in range(2):
                tpx = t_p.tile([BPC, 128], F32, tag="tp")
                nc.tensor.transpose(tpx, ptrT_sb[:, c], identF)
                nc.vector.tensor_scalar_mul(ptr[:, c * 128:(c + 1) * 128], tpx, prec)

            # ---- final MLP ----
            finT = cst.tile([128, 6, BPC], BF16, tag="finT")
            for piece, src in enumerate((menh, ptr, cur)):
                for c in range(2):
                    tp = t_p.tile([128, BPC], F32, tag="tp")
                    nc.tensor.transpose(tp, src[:, c * 128:(c + 1) * 128], i32f[:, :])
                    nc.scalar.activation(out=finT[:, piece * 2 + c], in_=tp, func=COPY)
            op1_sb = load2(op1_wT, 2 * D, "op1_sb", BF16, 6)
            op1b_sb = load2(op1_b, 1, "op1b_sb", F32, 4)
            hid = cst.tile([BPC, 2 * D], F32, tag="hid")
            for mc in range(4):
                ps = t_p.tile([128, BPC], F32, tag="tp")
                for kc in range(6):
                    nc.tensor.matmul(ps, lhsT=op1_sb[:, kc, ts(mc, 128)],
                                     rhs=finT[:, kc], start=(kc == 0), stop=(kc == 5))
                hT = sb.tile([128, BPC], F32, tag="hT")
                nc.scalar.activation(out=hT, in_=ps, func=GELU, bias=op1b_sb[:, mc])
                tp2 = t_p.tile([BPC, 128], F32, tag="tp")
                nc.tensor.transpose(tp2, hT, identF)
                nc.scalar.activation(out=hid[:, ts(mc, 128)], in_=tp2, func=COPY)
            nc.sync.dma_start(out=hid_o[:, :], in_=hid)
    nc.finalize()
    return nc


# ---------------------------------------------------------------------------
# Head program
# ---------------------------------------------------------------------------

def build_head():
    nc = bacc_mod.Bacc(None, target_bir_lowering=False, debug=True)
    hid = nc.dram_tensor("hid", [B, 2 * D], F32, kind="ExternalInput")
    op2_wTs = nc.dram_tensor("op2_wTs", [2 * D, RPAD], F32R, kind="ExternalInput")
    op2_bs = nc.dram_tensor("op2_bs", [1, RPAD], F32, kind="ExternalInput")
    mem = nc.dram_tensor("mem", [SLOTS, D], F32, kind="ExternalInput")
    ohwT = nc.dram_tensor("ohwT", [B, SLOTS], F32R, kind="ExternalInput")
    decay = nc.dram_tensor("decay", [SLOTS, 1], F32, kind="ExternalInput")
    cur = nc.dram_tensor("cur", [B, D], F32R, kind="ExternalInput")
    gate = nc.dram_tensor("gate", [B, 1], F32, kind="ExternalInput")
    logit_o = nc.dram_tensor("logit_o", [B, RPAD], F32, kind="ExternalOutput")
    mem_o = nc.dram_tensor("mem_o", [SLOTS, D], F32, kind="ExternalOutput")

    with TileContext(nc) as tc:
        with (tc.tile_pool(name="cst", bufs=1) as cst,
              tc.tile_pool(name="sb", bufs=3) as sb,
              tc.tile_pool(name="psp", bufs=3, space="PSUM") as psp,
              tc.tile_pool(name="tpp", bufs=2, space="PSUM") as tpp):
            ident = cst.tile([128, 128], F32, tag="ident")
            make_identity(nc, ident)
            hid_sb = cst.tile([128, 2, 2 * D], F32, tag="hid_sb")
            for c in range(2):
                nc.sync.dma_start(out=hid_sb[:, c], in_=hid[ts(c, 128), :])
            hidT = cst.tile([128, 4, B], F32R, tag="hidT")
            for kc in range(4):
                for c in range(2):
                    tp = tpp.tile([128, 128], F32, tag="tp")
                    nc.tensor.transpose(tp, hid_sb[:, c, ts(kc, 128)], ident)
                    nc.scalar.activation(out=hidT[:, kc, ts(c, 128)], in_=tp, func=COPY)
            w_sb = cst.tile([128, 4, RPAD], F32R, tag="w_sb")
            for c in range(4):
                nc.sync.dma_start(out=w_sb[:, c], in_=op2_wTs[ts(c, 128), :])
            ob_sb = cst.tile([128, RPAD], F32, tag="ob_sb")
            nc.sync.dma_start(out=ob_sb, in_=_bcast_ap(op2_bs[:, :]))
            for mc in range(2):
                for nch in range(3):
                    nw = 512 if nch < 2 else 256
                    ncol = ds(nch * 512, nw)
                    ps = psp.tile([128, 512], F32, tag="ps")
                    for kc in range(4):
                        nc.tensor.matmul(ps[:, 0:nw], lhsT=hidT[:, kc, ts(mc, 128)],
                                         rhs=w_sb[:, kc, ncol],
                                         start=(kc == 0), stop=(kc == 3))
                    lo = sb.tile([128, 512], F32, tag="lo")
                    nc.vector.tensor_add(lo[:, 0:nw], ps[:, 0:nw], ob_sb[:, ncol])
                    nc.sync.dma_start(out=logit_o[ts(mc, 128), ncol], in_=lo[:, 0:nw])
            # EMA
            cur_sb = cst.tile([128, 2, D], F32R, tag="cur_sb")
            gate_sb = cst.tile([128, 2, 1], F32, tag="gate_sb")
            for c in range(2):
                nc.sync.dma_start(out=cur_sb[:, c], in_=cur[ts(c, 128), :])
                nc.sync.dma_start(out=gate_sb[:, c], in_=gate[ts(c, 128), :])
            for c in range(2):
                nc.vector.tensor_scalar_mul(cur_sb[:, c], cur_sb[:, c], gate_sb[:, c])
            ohwT_sb = cst.tile([128, 2, SLOTS], F32R, tag="ohwT_sb")
            for c in range(2):
                nc.sync.dma_start(out=ohwT_sb[:, c], in_=ohwT[ts(c, 128), :])
            mem_sb = cst.tile([128, 4, D], F32, tag="mem_sb")
            dec_sb = cst.tile([128, 4, 1], F32, tag="dec_sb")
            for c in range(4):
                nc.sync.dma_start(out=mem_sb[:, c], in_=mem[ts(c, 128), :])
                nc.sync.dma_start(out=dec_sb[:, c], in_=decay[ts(c, 128), :])
            for sc in range(4):
                ps = psp.tile([128, D], F32, tag="ps")
                for kc in range(2):
                    nc.tensor.matmul(ps, lhsT=ohwT_sb[:, kc, ts(sc, 128)],
                                     rhs=cur_sb[:, kc], start=(kc == 0), stop=(kc == 1))
                nc.vector.tensor_scalar_mul(mem_sb[:, sc], mem_sb[:, sc], dec_sb[:, sc])
                nc.vector.tensor_add(mem_sb[:, sc], mem_sb[:, sc], ps)
                nc.sync.dma_start(out=mem_o[ts(sc, 128), :], in_=mem_sb[:, sc])
    nc.finalize()
    return nc


# ---------------------------------------------------------------------------
# Host orchestration
# ---------------------------------------------------------------------------

_CACHE = {}
EXEC_TIMES = []


def _get(name, builder, *a):
    if name not in _CACHE:
        _CACHE[name] = builder(*a)
    return _CACHE[name]


def _run(nc, in_maps):
    import time as _time
    t0 = _time.perf_counter()
    r = run_bass_kernel_spmd(nc, in_maps, CORE_IDS)
    EXEC_TIMES.append(int((_time.perf_counter() - t0) * 1e9))
    return r.results


def _pad_rows(a, n):
    out = np.zeros((n, a.shape[1]), np.float32)
    out[:a.shape[0]] = np.asarray(a, np.float32)
    return out


def kernel(locations, users, start_mins, weekdays, target, base_embed, adj_left,
           adj_right, gc1_w, gc1_b, gc2_w, gc2_b, user_emb, hour_emb, wd_emb,
           memory, rq_w, rq_b, rk_w, rk_b, wg_w, wg_b, pq_w, pq_b, pk_w, pk_b,
           pv_w, pv_b, wih0, whh0, bih0, bhh0, wih1, whh1, bih1, bhh1, ain_w,
           ain_b, aout_w, aout_b, n1g, n1b, n2g, n2b, n3g, n3b, op1_w, op1_b,
           op2_w, op2_b):
    f32 = np.float32
    A = lambda v: np.ascontiguousarray(v, f32)
    row = lambda v: A(np.reshape(v, (1, -1)))
    col = lambda v: A(np.reshape(v, (-1, 1)))
    adj_leftT = A(np.asarray(adj_left).T)

    hop1 = _get("hop1", build_hop, True)
    hop2 = _get("hop2", build_hop, False)
    in1 = []
    for c in range(NC):
        lb = np.zeros((RANK, RPAD), f32)
        lb[:, :RPC] = adj_leftT[:, c * RPC:(c + 1) * RPC]
        in1.append(dict(lbT=lb, radj=A(adj_right), xin=_pad_rows(np.asarray(base_embed), KPAD),
                        gw=A(np.asarray(gc1_w).T), gb=row(gc1_b)))
    r1 = _run(hop1, in1)
    nb1 = np.concatenate([r1[c]["out"][:RPC] for c in range(NC)], 0)

    in2 = [dict(lbT=in1[c]["lbT"], radj=in1[c]["radj"], xin=_pad_rows(nb1, KCH * 128),
                gw=A(np.asarray(gc2_w).T), gb=row(gc2_b),
                recip_i=r1[c]["recip_o"],
                basei=_pad_rows(np.asarray(base_embed)[c * RPC:(c + 1) * RPC], RPAD))
           for c in range(NC)]
    r2 = _run(hop2, in2)
    Y = np.concatenate([r2[c]["out"][:RPC] for c in range(NC)], 0)

    hours = (np.asarray(start_mins) // 60) % 24
    x = (Y[np.asarray(locations)]
         + np.asarray(user_emb)[np.asarray(users)[:, 0]][:, None, :]
         + np.asarray(hour_emb)[hours]
         + np.asarray(wd_emb)[np.asarray(weekdays)]).astype(f32)

    affine = not all(np.allclose(g, 1) and np.allclose(bb, 0)
                     for g, bb in ((n1g, n1b), (n2g, n2b), (n3g, n3b)))
    seq = _get(f"seq{affine}", build_seq, affine)
    bf = ml_dtypes.bfloat16
    common = dict(
        wihT0=np.ascontiguousarray(np.asarray(wih0, f32).T).astype(bf),
        whhT0=np.ascontiguousarray(np.asarray(whh0, f32).T).astype(bf),
        wihT1=np.ascontiguousarray(np.asarray(wih1, f32).T).astype(bf),
        whhT1=np.ascontiguousarray(np.asarray(whh1, f32).T).astype(bf),
        bias0=row(np.asarray(bih0) + np.asarray(bhh0)),
        bias1=row(np.asarray(bih1) + np.asarray(bhh1)),
        ain_wT=A(np.asarray(ain_w).T[:, 0:2 * D]),
        av_wT=A(np.asarray(ain_w).T[:, 2 * D:3 * D]),
        ain_qkb=col(np.asarray(ain_b)[0:2 * D]), ain_vb=row(np.asarray(ain_b)[2 * D:]),
        aout_wT=A(np.asarray(aout_w).T), aout_b=row(aout_b),
        n1g=row(n1g), n1b=row(n1b), n2g=row(n2g), n2b=row(n2b),
        n3g=row(n3g), n3b=row(n3b),
        rq_wT=A(np.asarray(rq_w).T), rq_b=col(rq_b),
        rk_wT=A(np.asarray(rk_w).T), rk_b=col(rk_b),
        memT=A(np.asarray(memory).T),
        mem_aug=A(np.concatenate([np.asarray(memory, f32),
                                  np.ones((SLOTS, 1), f32),
                                  np.zeros((SLOTS, 1), f32)], 1)),
        wg_wT=A(np.asarray(wg_w).T), wg_b=A(np.asarray(wg_b)).reshape(1, 1),
        pq_wT=A(np.asarray(pq_w).T), pq_b=col(pq_b),
        pk_wT=A(np.asarray(pk_w).T), pk_b=col(pk_b),
        pv_wT=A(np.asarray(pv_w).T), pv_b=row(pv_b),
        op1_wT=np.ascontiguousarray(np.asarray(op1_w, f32).T).astype(bf),
        op1_b=col(op1_b),
        onesc=np.ones((128, 1), f32).astype(bf),
    )
    in3 = []
    for c in range(NC):
        xb = x[c * BPC:(c + 1) * BPC]
        in3.append(dict(common, x_bp=A(xb.reshape(NT, D)),
                        xT=np.ascontiguousarray(
                            xb.transpose(2, 1, 0).reshape(D, 4, 16, BPC)
                            .transpose(0, 2, 1, 3).reshape(D, S, BPC)).astype(bf)))
    r3 = _run(seq, in3)
    hid = np.concatenate([r3[c]["hid_o"] for c in range(NC)], 0)
    cur = np.concatenate([r3[c]["cur_o"] for c in range(NC)], 0)
    gate = np.concatenate([r3[c]["gate_o"][0] for c in range(NC)], 0)

    addr = (np.asarray(target) % SLOTS).astype(np.int64)
    later = np.zeros(B, f32)
    count = np.zeros(SLOTS, np.int64)
    for b in range(B - 1, -1, -1):
        later[b] = count[addr[b]]
        count[addr[b]] += 1
    coeff = (0.1 * np.power(0.9, later)).astype(f32)
    decay = np.power(0.9, count.astype(f32)).astype(f32)
    ohwT = np.zeros((B, SLOTS), f32)
    ohwT[np.arange(B), addr] = coeff

    head = _get("head", build_head)
    op2_wT = A(np.asarray(op2_w).T)
    in4 = []
    for c in range(NC):
        ws = np.zeros((2 * D, RPAD), f32)
        ws[:, :RPC] = op2_wT[:, c * RPC:(c + 1) * RPC]
        bs = np.zeros((1, RPAD), f32)
        bs[0, :RPC] = np.asarray(op2_b, f32)[c * RPC:(c + 1) * RPC]
        in4.append(dict(hid=hid, op2_wTs=ws, op2_bs=bs, mem=A(memory), ohwT=ohwT,
                        decay=col(decay), cur=cur, gate=col(gate)))
    r4 = _run(head, in4)
    logits = np.concatenate([r4[c]["logit_o"][:, :RPC] for c in range(NC)], 1)
    new_mem = r4[0]["mem_o"]
    return logits, new_mem


# revision 28
# speedup vs baseline: 1.3284x; 1.3284x over previous
import sys
sys.path.insert(0, '/opt/trn_rl_repo')
import numpy as np
import ml_dtypes

import concourse.bass as bass
import concourse.bacc as bacc_mod
import concourse.mybir as mybir
from concourse.tile import TileContext
from concourse.masks import make_identity
from concourse.bass_utils import run_bass_kernel_spmd

F32 = mybir.dt.float32
F32R = mybir.dt.float32r
BF16 = mybir.dt.bfloat16
EXP = mybir.ActivationFunctionType.Exp
SIG = mybir.ActivationFunctionType.Sigmoid
TANH = mybir.ActivationFunctionType.Tanh
RELU = mybir.ActivationFunctionType.Relu
GELU = mybir.ActivationFunctionType.Gelu
COPY = mybir.ActivationFunctionType.Copy
IDNT = mybir.ActivationFunctionType.Identity
SQRT = mybir.ActivationFunctionType.Sqrt
AXX = mybir.AxisListType.X
ts, ds = bass.ts, bass.ds

B, S, NL, NU, D, SLOTS, RANK, H = 256, 64, 10000, 2000, 256, 512, 50, 4
NC = 8
RPC, RPAD = 1250, 1280
BLK, NBLK = 256, 5
KCH = (NL + 127) // 128        # 79
KPAD = KCH * 128
BPC = B // NC                  # 32
NT = BPC * S                   # 2048
NTC = NT // 128                # 16
G4 = 4 * D                     # 1024
DH = D // H                    # 64
ISD = 1.0 / np.sqrt(D)
ISH = 1.0 / np.sqrt(DH)
CORE_IDS = list(range(NC))


def _bcast_ap(dram_ap, p=128):
    return bass.AP(tensor=dram_ap.tensor, offset=dram_ap.offset,
                   ap=[[0, p]] + [list(x) for x in dram_ap.ap[1:]])


# ---------------------------------------------------------------------------
# Graph hop
# ---------------------------------------------------------------------------

def build_hop(hop1: bool):
    nc = bacc_mod.Bacc(None, target_bir_lowering=False, debug=True)
    lbT = nc.dram_tensor("lbT", [RANK, RPAD], F32R, kind="ExternalInput")
    radj = nc.dram_tensor("radj", [RANK, NL], F32R, kind="ExternalInput")
    xin = nc.dram_tensor("xin", [KCH * 128, D], BF16, kind="ExternalInput")
    gw = nc.dram_tensor("gw", [D, D], F32R, kind="ExternalInput")
    gb = nc.dram_tensor("gb", [1, D], F32, kind="ExternalInput")
    out = nc.dram_tensor("out", [RPAD, D], F32, kind="ExternalOutput")
    if hop1:
        recip_o = nc.dram_tensor("recip_o", [RPAD, 1], F32, kind="ExternalOutput")
    else:
        recip_i = nc.dram_tensor("recip_i", [RPAD, 1], F32, kind="ExternalInput")
        basei = nc.dram_tensor("basei", [RPAD, D], F32, kind="ExternalInput")

    with TileContext(nc) as tc:
        with (tc.tile_pool(name="const", bufs=1) as cst,
              tc.tile_pool(name="sb", bufs=3) as sb,
              tc.tile_pool(name="sb2", bufs=2) as sb2,
              tc.tile_pool(name="eps_p", bufs=2, space="PSUM") as eps_p,
              tc.tile_pool(name="z_p", bufs=1, space="PSUM") as z_p,
              tc.tile_pool(name="tp_p", bufs=2, space="PSUM") as tp_p,
              tc.tile_pool(name="nb_p", bufs=2, space="PSUM") as nb_p):
            lbT_sb = cst.tile([RANK, RPAD], F32R, tag="lbT")
            nc.sync.dma_start(out=lbT_sb, in_=lbT[:, :])
            radj_sb = cst.tile([RANK, NL], F32R, tag="radj")
            nc.sync.dma_start(out=radj_sb, in_=radj[:, :])
            xin_sb = cst.tile([128, KCH * D], BF16, tag="xin")
            nc.sync.dma_start(out=xin_sb,
                              in_=xin.rearrange("(k p) d -> p k d", p=128))
            gw_sb = cst.tile([128, 2, D], F32R, tag="gw")
            nc.sync.dma_start(out=gw_sb[:, 0, :], in_=gw[0:128, :])
            nc.sync.dma_start(out=gw_sb[:, 1, :], in_=gw[128:256, :])
            gbb = cst.tile([128, D], F32, tag="gbb")
            nc.sync.dma_start(out=gbb, in_=_bcast_ap(gb[:, :]))
            ident = cst.tile([128, 128], F32, tag="ident")
            make_identity(nc, ident)

            for m in range(NBLK):
                rows = ts(m, BLK)
                zA = z_p.tile([128, BLK], F32, tag="zA")
                zB = z_p.tile([128, BLK], F32, tag="zB")
                eacc = sb.tile([128, BLK], F32, tag="eacc")
                if hop1:
                    nc.vector.memset(eacc, 0.0)
                for k in range(KCH):
                    pk = min(128, NL - k * 128)
                    eps = eps_p.tile([128, BLK], F32, tag="eps")
                    nc.tensor.matmul(eps[:pk], lhsT=radj_sb[:, k * 128:k * 128 + pk],
                                     rhs=lbT_sb[:, rows], start=True, stop=True)
                    e = sb.tile([128, BLK], BF16, tag="e")
                    nc.scalar.activation(out=e[:pk], in_=eps[:pk], func=EXP)
                    nc.tensor.matmul(zA, lhsT=xin_sb[:pk, k * D:k * D + 128],
                                     rhs=e[:pk], start=(k == 0), stop=(k == KCH - 1))
                    nc.tensor.matmul(zB, lhsT=xin_sb[:pk, k * D + 128:k * D + 256],
                                     rhs=e[:pk], start=(k == 0), stop=(k == KCH - 1))
                    if hop1:
                        nc.vector.tensor_add(eacc[:pk], eacc[:pk], e[:pk])
                zsA = sb2.tile([128, BLK], F32R, tag="zsA")
                nc.scalar.activation(out=zsA, in_=zA, func=COPY)
                zsB = sb2.tile([128, BLK], F32R, tag="zsB")
                nc.scalar.activation(out=zsB, in_=zB, func=COPY)
                for h in range(2):
                    rh = ds(m * BLK + h * 128, 128)
                    recip = sb.tile([128, 1], F32, tag="recip")
                    if hop1:
                        tp = tp_p.tile([128, 128], F32, tag="tp")
                        nc.tensor.transpose(tp, eacc[:, h * 128:(h + 1) * 128], ident)
                        rsum = sb.tile([128, 1], F32, tag="rsum")
                        nc.vector.reduce_sum(out=rsum, in_=tp, axis=AXX)
                        nc.vector.reciprocal(out=recip, in_=rsum)
                        nc.sync.dma_start(out=recip_o[rh, :], in_=recip)
                    else:
                        nc.sync.dma_start(out=recip, in_=recip_i[rh, :])
                    nb = nb_p.tile([128, D], F32, tag="nb")
                    nc.tensor.matmul(nb, lhsT=zsA[:, h * 128:(h + 1) * 128],
                                     rhs=gw_sb[:, 0, :], start=True, stop=False)
                    nc.tensor.matmul(nb, lhsT=zsB[:, h * 128:(h + 1) * 128],
                                     rhs=gw_sb[:, 1, :], start=False, stop=True)
                    nc.vector.tensor_scalar_mul(nb, nb, recip)
                    nc.vector.tensor_add(nb, nb, gbb)
                    ot = sb.tile([128, D], F32, tag="ot")
                    if hop1:
                        nc.scalar.activation(out=ot, in_=nb, func=RELU)
                    else:
                        bt = sb.tile([128, D], F32, tag="bt")
                        nc.sync.dma_start(out=bt, in_=basei[rh, :])
                        nc.vector.tensor_add(ot, nb, bt)
                    nc.sync.dma_start(out=out[rh, :], in_=ot)
    nc.finalize()
    return nc


# ---------------------------------------------------------------------------
# Sequence program
# ---------------------------------------------------------------------------

def build_seq(affine: bool):
    nc = bacc_mod.Bacc(None, target_bir_lowering=False, debug=True)

    def I(name, shape, dt=F32):
        return nc.dram_tensor(name, shape, dt, kind="ExternalInput")

    x_bp = I("x_bp", [NT, D])
    xT = I("xT", [D, S, BPC], BF16)
    wihT0 = I("wihT0", [D, G4], BF16)
    whhT0 = I("whhT0", [D, G4], BF16)
    whhT1 = I("whhT1", [D, G4], BF16)
    wihT1 = I("wihT1", [D, G4], BF16)
    bias0 = I("bias0", [1, G4])
    bias1 = I("bias1", [1, G4])
    ain_wT = I("ain_wT", [D, 2 * D], F32R)
    av_wT = I("av_wT", [D, D], F32R)
    ain_qkb = I("ain_qkb", [2 * D, 1])
    ain_vb = I("ain_vb", [1, D])
    aout_wT = I("aout_wT", [D, D], F32R)
    aout_b = I("aout_b", [1, D])
    lng = {}; lnb = {}
    for j in (1, 2, 3):
        lng[j] = I(f"n{j}g", [1, D]); lnb[j] = I(f"n{j}b", [1, D])
    rq_wT = I("rq_wT", [D, D], F32R); rq_b = I("rq_b", [D, 1])
    rk_wT = I("rk_wT", [D, D], F32R); rk_b = I("rk_b", [D, 1])
    memT = I("memT", [D, SLOTS], F32R)
    mem_aug = I("mem_aug", [SLOTS, D + 2], F32R)
    wg_wT = I("wg_wT", [D, 1], F32R); wg_b = I("wg_b", [1, 1])
    pq_wT = I("pq_wT", [D, D], F32R); pq_b = I("pq_b", [D, 1])
    pk_wT = I("pk_wT", [D, D], F32R); pk_b = I("pk_b", [D, 1])
    pv_wT = I("pv_wT", [D, D], F32R); pv_b = I("pv_b", [1, D])
    op1_wT = I("op1_wT", [3 * D, 2 * D], BF16)
    op1_b = I("op1_b", [2 * D, 1])
    onesc = I("onesc", [128, 1], BF16)
    hid_o = nc.dram_tensor("hid_o", [BPC, 2 * D], F32, kind="ExternalOutput")
    cur_o = nc.dram_tensor("cur_o", [BPC, D], F32, kind="ExternalOutput")
    gate_o = nc.dram_tensor("gate_o", [1, BPC], F32, kind="ExternalOutput")

    with TileContext(nc) as tc:
        with (tc.tile_pool(name="cst", bufs=1) as cst,
              tc.tile_pool(name="bigp", bufs=1) as bigp,
              tc.tile_pool(name="sb", bufs=3) as sb,
              tc.tile_pool(name="ps4", bufs=2, space="PSUM") as ps4,
              tc.tile_pool(name="g_p", bufs=1, space="PSUM") as g_p,
              tc.tile_pool(name="t_p", bufs=2, space="PSUM") as t_p):

            def bc(ap, tag, n=D, p=128, dt=F32):
                t = cst.tile([p, n], dt, tag=tag, name=tag)
                nc.gpsimd.dma_start(out=t, in_=_bcast_ap(ap[:, :], p))
                return t

            def load2(dr, n, tag, dt=F32R, chunks=2):
                t = cst.tile([128, chunks, n], dt, tag=tag, name=tag)
                for c in range(chunks):
                    nc.gpsimd.dma_start(out=t[:, c], in_=dr[c * 128:(c + 1) * 128])
                return t

            identF = cst.tile([128, 128], F32, tag="identF")
            make_identity(nc, identF)
            i32stk = cst.tile([64, 32], BF16, tag="i32stk")
            for j in range(2):
                make_identity(nc, i32stk[32 * j:32 * (j + 1), :])
            i32f = cst.tile([32, 32], F32, tag="i32f")
            make_identity(nc, i32f)
            eps_sb = cst.tile([128, 1], F32, tag="eps_sb")
            nc.vector.memset(eps_sb, 1e-5)

            ones_sb = cst.tile([128, 1], BF16, tag="ones_sb")
            nc.sync.dma_start(out=ones_sb, in_=onesc[:, :])
            xT_sb = bigp.tile([128, 2, S * BPC], BF16, tag="bigE", name="xT_sb")
            for c in range(2):
                nc.sync.dma_start(out=xT_sb[:, c], in_=xT[c * 128:(c + 1) * 128])
            x_bp_sb = bigp.tile([128, NTC, D], F32, tag="bigB", name="x_bp_sb")
            for i in range(NTC):
                nc.sync.dma_start(out=x_bp_sb[:, i, :], in_=x_bp[ts(i, 128), :])
            wih0_sb = load2(wihT0, G4, "wih0_sb", BF16)
            whh0_sb = load2(whhT0, G4, "whh0_sb", BF16)
            whh1_sb = load2(whhT1, G4, "whh1_sb", BF16)
            wih1_sb = load2(wihT1, G4, "wih1_sb", BF16)
            b0_full = bc(bias0, "b0_full", G4)
            bias1_b32 = bc(bias1, "bias1_b32", G4, 32, BF16)

            # ---- xg1 = x @ wihT0 (+bias0), tg-packed, bf16 ----
            xg1 = bigp.tile([64, 16, G4], BF16, tag="bigA", name="xg1")
            xv = xT_sb.rearrange("p c (tsub tg b) -> p c tsub tg b", tsub=16, b=BPC)

            def emit_xg_wave(w):
                for tsub in range(16):
                    for nh in range(2):
                        ps = ps4.tile([64, 512], F32, tag="ps")
                        for kc in range(2):
                            nc.tensor.matmul(
                                ps, lhsT=xv[:, kc, tsub, 2 * w:2 * w + 2, :],
                                rhs=wih0_sb[:, kc, ts(nh, 512)],
                                start=(kc == 0), stop=(kc == 1))
                        if tsub % 2 == 0:
                            nc.scalar.activation(out=xg1[:, tsub, ts(nh, 512)],
                                                 in_=ps, func=COPY)
                        else:
                            nc.vector.tensor_copy(out=xg1[:, tsub, ts(nh, 512)],
                                                  in_=ps)
                    nc.vector.tensor_add(xg1[:, tsub, :], xg1[:, tsub, :],
                                         b0_full[0:64])

            # ---- LSTM ----
            h1T = cst.tile([128, 2, BPC], BF16, tag="h1T")
            h2T_all = bigp.tile([128, 2, BPC, S], BF16, tag="bigC", name="h2T_all")
            c1 = cst.tile([BPC, D], F32, tag="c1")
            c2 = cst.tile([BPC, D], F32, tag="c2")
            nc.vector.memset(h1T, 0.0)
            nc.vector.memset(c1, 0.0)
            nc.vector.memset(c2, 0.0)

            def lstm_step(t, layer):
                glo = g_p.tile([BPC, 512], F32, tag="glo")
                ghi = g_p.tile([BPC, 512], F32, tag="ghi")
                if layer == 1:
                    tg, tsub = t // 16, t % 16
                    tg1 = tg % 2
                    xs = xg1[tg1 * 32:(tg1 + 1) * 32, tsub]
                    ii = i32stk[tg1 * 32:(tg1 + 1) * 32, :]
                    nc.tensor.matmul(glo, lhsT=ii, rhs=xs[:, 0:512],
                                     start=True, stop=False)
                    nc.tensor.matmul(ghi, lhsT=ii, rhs=xs[:, 512:1024],
                                     start=True, stop=False)
                    for kc in range(2):
                        nc.tensor.matmul(glo, lhsT=h1T[:, kc],
                                         rhs=whh0_sb[:, kc, 0:512],
                                         start=False, stop=(kc == 1))
                        nc.tensor.matmul(ghi, lhsT=h1T[:, kc],
                                         rhs=whh0_sb[:, kc, 512:1024],
                                         start=False, stop=(kc == 1))
                else:
                    nc.tensor.matmul(glo, lhsT=i32stk[0:32, :], rhs=bias1_b32[:, 0:512],
                                     start=True, stop=False)
                    nc.tensor.matmul(ghi, lhsT=i32stk[0:32, :], rhs=bias1_b32[:, 512:1024],
                                     start=True, stop=False)
                    for kc in range(2):
                        last = (kc == 1)
                        if t > 0:
                            nc.tensor.matmul(glo, lhsT=h2T_all[:, kc, :, t - 1],
                                             rhs=whh1_sb[:, kc, 0:512],
                                             start=False, stop=False)
                            nc.tensor.matmul(ghi, lhsT=h2T_all[:, kc, :, t - 1],
                                             rhs=whh1_sb[:, kc, 512:1024],
                                             start=False, stop=False)
                        nc.tensor.matmul(glo, lhsT=h1T[:, kc],
                                         rhs=wih1_sb[:, kc, 0:512],
                                         start=False, stop=last)
                        nc.tensor.matmul(ghi, lhsT=h1T[:, kc],
                                         rhs=wih1_sb[:, kc, 512:1024],
                                         start=False, stop=last)
                cprev = c1 if layer == 1 else c2
                sif = sb.tile([BPC, 512], F32, tag="sif")
                nc.scalar.activation(out=sif, in_=glo, func=SIG)
                tg_ = sb.tile([BPC, D], F32, tag="tg_")
                nc.scalar.activation(out=tg_, in_=ghi[:, 0:D], func=TANH)
                so = sb.tile([BPC, D], F32, tag="so")
                nc.scalar.activation(out=so, in_=ghi[:, D:2 * D], func=SIG)
                nc.vector.tensor_mul(cprev, sif[:, 256:512], cprev)
                ig = sb.tile([BPC, D], F32, tag="ig")
                nc.vector.tensor_mul(ig, sif[:, 0:256], tg_)
                nc.vector.tensor_add(cprev, cprev, ig)
                tc_ = sb.tile([BPC, D], F32, tag="tc_")
                nc.scalar.activation(out=tc_, in_=cprev, func=TANH)
                hnew = sb.tile([BPC, D], F32, tag="hnew")
                nc.vector.tensor_mul(hnew, so, tc_)
                for c in range(2):
                    tp = t_p.tile([128, BPC], F32, tag="tp")
                    nc.tensor.transpose(tp, hnew[:, c * 128:(c + 1) * 128],
                                        i32f[:, :])
                    dst = h1T[:, c] if layer == 1 else h2T_all[:, c, :, t]
                    nc.scalar.activation(out=dst, in_=tp, func=COPY)

            emit_xg_wave(0)
            for t in range(S):
                if t == 32:
                    emit_xg_wave(1)
                lstm_step(t, 1)
                lstm_step(t, 2)

            # ---- res1 + LN1 ----
            ng = {j: bc(lng[j][:, :], f"n{j}g_b") for j in (1, 2, 3)} if affine else {}
            nb_ = {j: bc(lnb[j][:, :], f"n{j}b_b") for j in (1, 2, 3)} if affine else {}

            def ln_rows(xi, j, p=128):
                stats = sb.tile([p, 6], F32, tag="lnst", name="lnst")
                nc.vector.bn_stats(out=stats, in_=xi)
                mv = sb.tile([p, 2], F32, tag="lnmv", name="lnmv")
                nc.vector.bn_aggr(out=mv, in_=stats)
                std = sb.tile([p, 1], F32, tag="lnsd", name="lnsd")
                nc.scalar.activation(out=std, in_=mv[:, 1:2], func=SQRT,
                                     bias=eps_sb[:p, :])
                rstd = sb.tile([p, 1], F32, tag="lnrs", name="lnrs")
                nc.vector.reciprocal(out=rstd, in_=std)
                nmu = sb.tile([p, 1], F32, tag="lnnm", name="lnnm")
                nc.vector.tensor_mul(nmu, mv[:, 0:1], rstd)
                nc.vector.tensor_scalar_mul(nmu, nmu, -1.0)
                nc.vector.tensor_scalar_mul(xi, xi, rstd)
                nc.vector.tensor_scalar_add(xi, xi, nmu)
                if affine:
                    nc.vector.tensor_mul(xi, xi, ng[j][:p])
                    nc.vector.tensor_add(xi, xi, nb_[j][:p])

            lout_bp = bigp.tile([128, NTC, D], F32, tag="bigD", name="lout_bp")
            for i in range(NTC):
                for c in range(2):
                    tt = sb.tile([128, 128], BF16, tag="dmat")
                    nc.sync.dma_start_transpose(out=tt,
                                                in_=h2T_all[:, c, 2 * i:2 * i + 2, :])
                    nc.vector.tensor_add(lout_bp[:, i, c * 128:(c + 1) * 128],
                                         tt, x_bp_sb[:, i, c * 128:(c + 1) * 128])
            for i in range(NTC):
                ln_rows(lout_bp[:, i, :], 1)

            # ---- qkT, v_bp ----
            loutT = bigp.tile([128, 2, NT], F32R, tag="bigA", name="loutT")
            for i in range(NTC):
                for c in range(2):
                    tp = t_p.tile([128, 128], F32, tag="tp")
                    nc.tensor.transpose(tp, lout_bp[:, i, c * 128:(c + 1) * 128], identF)
                    nc.scalar.activation(out=loutT[:, c, ts(i, 128)], in_=tp, func=COPY)
            ain_sb = load2(ain_wT, 2 * D, "ain_sb")
            aqkb_sb = load2(ain_qkb, 1, "aqkb_sb", F32, 4)
            qkT = bigp.tile([128, 4, NT], BF16, tag="bigB", name="qkT")
            for mc in range(4):
                for nch in range(4):
                    ps = ps4.tile([128, 512], F32, tag="ps")
                    for kc in range(2):
                        nc.tensor.matmul(ps, lhsT=ain_sb[:, kc, ts(mc, 128)],
                                         rhs=loutT[:, kc, ts(nch, 512)],
                                         start=(kc == 0), stop=(kc == 1))
                    nc.scalar.activation(out=qkT[:, mc, ts(nch, 512)], in_=ps,
                                         func=IDNT, bias=aqkb_sb[:, mc])
            av_sb = load2(av_wT, D, "av_sb")
            avb_b = bc(ain_vb[:, :], "avb_b")
            v_bp = bigp.tile([128, NTC, D], BF16, tag="bigC", name="v_bp")
            for i in range(NTC):
                ps = ps4.tile([128, D], F32, tag="ps")
                for kc in range(2):
                    nc.tensor.matmul(ps, lhsT=loutT[:, kc, ts(i, 128)],
                                     rhs=av_sb[:, kc], start=(kc == 0), stop=(kc == 1))
                nc.vector.tensor_add(v_bp[:, i, :], ps, avb_b)

            # ---- attention ----
            attn_bp = bigp.tile([128, NTC, D], F32, tag="bigE", name="attn_bp")
            for b in range(BPC):
                i, r0 = b // 2, (b % 2) * 64
                vb = v_bp[r0:r0 + 64, i]
                op = t_p.tile([64, 264], F32, tag="op", bufs=1)
                for hh in range(H):
                    qh = qkT[(hh % 2) * 64:(hh % 2) * 64 + 64, hh // 2,
                             b * S:(b + 1) * S]
                    kh = qkT[(hh % 2) * 64:(hh % 2) * 64 + 64, 2 + hh // 2,
                             b * S:(b + 1) * S]
                    sT = t_p.tile([128, 64], F32, tag="tp")
                    nc.tensor.matmul(sT[r0:r0 + 64], lhsT=kh, rhs=qh,
                                     start=True, stop=True)
                    eT = sb.tile([128, 64], BF16, tag="eT")
                    nc.scalar.activation(out=eT[r0:r0 + 64], in_=sT[r0:r0 + 64],
                                         func=EXP, scale=ISH)
                    nc.tensor.matmul(op[:, hh * 64:(hh + 1) * 64],
                                     lhsT=eT[r0:r0 + 64],
                                     rhs=vb[:, hh * 64:(hh + 1) * 64],
                                     start=True, stop=True)
                    nc.tensor.matmul(op[:, 256 + hh:257 + hh],
                                     lhsT=eT[r0:r0 + 64],
                                     rhs=ones_sb[r0:r0 + 64, :],
                                     start=True, stop=True)
                rec4 = sb.tile([64, 4], F32, tag="rec4")
                nc.vector.reciprocal(out=rec4, in_=op[:, 256:260])
                for hh in range(H):
                    nc.vector.tensor_scalar_mul(
                        attn_bp[r0:r0 + 64, i, hh * 64:(hh + 1) * 64],
                        op[:, hh * 64:(hh + 1) * 64], rec4[:, hh:hh + 1])

            # ---- proj + residual + LN2 ----
            attnT = bigp.tile([128, 2, NT], F32R, tag="bigA", name="attnT")
            for i in range(NTC):
                for c in range(2):
                    tp = t_p.tile([128, 128], F32, tag="tp")
                    nc.tensor.transpose(tp, attn_bp[:, i, c * 128:(c + 1) * 128], identF)
                    nc.scalar.activation(out=attnT[:, c, ts(i, 128)], in_=tp, func=COPY)
            aout_sb = load2(aout_wT, D, "aout_sb")
            aout_b_b = bc(aout_b[:, :], "aout_b_b")
            for i in range(NTC):
                ps = ps4.tile([128, D], F32, tag="ps")
                for kc in range(2):
                    nc.tensor.matmul(ps, lhsT=attnT[:, kc, ts(i, 128)],
                                     rhs=aout_sb[:, kc], start=(kc == 0), stop=(kc == 1))
                nc.vector.tensor_add(ps, ps, aout_b_b)
                nc.vector.tensor_add(lout_bp[:, i, :], lout_bp[:, i, :], ps)
            for i in range(NTC):
                ln_rows(lout_bp[:, i, :], 2)
            # lout_bp is now attn_out

            # ---- cur, curT ----
            cur = cst.tile([BPC, D], F32, tag="cur")
            nc.sync.dma_start(out=cur[0::2, :], in_=lout_bp[63:64, :, :])
            nc.sync.dma_start(out=cur[1::2, :], in_=lout_bp[127:128, :, :])
            curT = cst.tile([128, 2, BPC], BF16, tag="curT")
            for c in range(2):
                tp = t_p.tile([128, BPC], F32, tag="tp")
                nc.tensor.transpose(tp, cur[:, c * 128:(c + 1) * 128], i32f[:, :])
                nc.scalar.activation(out=curT[:, c], in_=tp, func=COPY)
            nc.sync.dma_start(out=cur_o[:, :], in_=cur)

            def matvecT(wT_dr, b_dr, tag):
                wsb = load2(wT_dr, D, f"w_{tag}", BF16)
                bsb = load2(b_dr, 1, f"b_{tag}", F32)
                res = cst.tile([128, 2, BPC], BF16, tag=f"r_{tag}", name=f"r_{tag}")
                for mc in range(2):
                    ps = t_p.tile([128, BPC], F32, tag="tp")
                    for kc in range(2):
                        nc.tensor.matmul(ps, lhsT=wsb[:, kc, ts(mc, 128)],
                                         rhs=curT[:, kc], start=(kc == 0), stop=(kc == 1))
                    nc.scalar.activation(out=res[:, mc], in_=ps, func=IDNT,
                                         bias=bsb[:, mc])
                return res

            # ---- memory read + LN3 -> menh ----
            mqT = matvecT(rq_wT, rq_b, "rq")
            rk_sb = load2(rk_wT, D, "rk_sb")  # stays f32r: N=512 fine
            rkb_sb = load2(rk_b, 1, "rkb_sb", F32)
            memT_sb = load2(memT, SLOTS, "memT_sb")
            mkT = cst.tile([128, 2, SLOTS], BF16, tag="mkT")
            for mc in range(2):
                ps = ps4.tile([128, SLOTS], F32, tag="ps")
                for kc in range(2):
                    nc.tensor.matmul(ps, lhsT=rk_sb[:, kc, ts(mc, 128)],
                                     rhs=memT_sb[:, kc], start=(kc == 0), stop=(kc == 1))
                nc.scalar.activation(out=mkT[:, mc], in_=ps, func=IDNT,
                                     bias=rkb_sb[:, mc])
            maug_sb = load2(mem_aug, D + 2, "maug_sb", F32R, 4)
            expT = cst.tile([128, 4, BPC], F32R, tag="expT")
            for sc in range(4):
                ps = t_p.tile([128, BPC], F32, tag="tp")
                for kc in range(2):
                    nc.tensor.matmul(ps, lhsT=mkT[:, kc, ts(sc, 128)],
                                     rhs=mqT[:, kc], start=(kc == 0), stop=(kc == 1))
                nc.scalar.activation(out=expT[:, sc], in_=ps, func=EXP, scale=ISD)
            mrd = t_p.tile([BPC, D + 2], F32, tag="tp")
            for sc in range(4):
                nc.tensor.matmul(mrd, lhsT=expT[:, sc], rhs=maug_sb[:, sc],
                                 start=(sc == 0), stop=(sc == 3))
            mrec = sb.tile([BPC, 1], F32, tag="mrec")
            nc.vector.reciprocal(out=mrec, in_=mrd[:, D:D + 1])
            menh = cst.tile([BPC, D], F32, tag="menh")
            nc.vector.tensor_scalar_mul(menh, mrd[:, 0:D], mrec)
            nc.vector.tensor_add(menh, menh, cur)
            ln_rows(menh, 3, BPC)

            # ---- gate ----
            wg_sb = load2(wg_wT, 1, "wg_sb", BF16)
            wgb_sb = cst.tile([1, 1], F32, tag="wgb_sb")
            nc.sync.dma_start(out=wgb_sb, in_=wg_b[:, :])
            gps = t_p.tile([1, BPC], F32, tag="tp")
            for kc in range(2):
                nc.tensor.matmul(gps, lhsT=wg_sb[:, kc], rhs=curT[:, kc],
                                 start=(kc == 0), stop=(kc == 1))
            gate_sb = sb.tile([1, BPC], F32, tag="gate")
            nc.scalar.activation(out=gate_sb, in_=gps, func=SIG, bias=wgb_sb[:, :])
            nc.sync.dma_start(out=gate_o[:, :], in_=gate_sb)

            # ---- pointer ----
            pqT = matvecT(pq_wT, pq_b, "pq")
            # post-LN2 transpose for pk/pv
            aoT = bigp.tile([128, 2, NT], F32R, tag="bigA", name="aoT")
            for i in range(NTC):
                for c in range(2):
                    tp = t_p.tile([128, 128], F32, tag="tp")
                    nc.tensor.transpose(tp, lout_bp[:, i, c * 128:(c + 1) * 128], identF)
                    nc.scalar.activation(out=aoT[:, c, ts(i, 128)], in_=tp, func=COPY)
            pk_sb = load2(pk_wT, D, "pk_sb")
            pkb_sb = load2(pk_b, 1, "pkb_sb", F32)
            pkT = bigp.tile([128, 2, NT], BF16, tag="bigB", name="pkT")
            for mc in range(2):
                for nch in range(4):
                    ps = ps4.tile([128, 512], F32, tag="ps")
                    for kc in range(2):
                        nc.tensor.matmul(ps, lhsT=pk_sb[:, kc, ts(mc, 128)],
                                         rhs=aoT[:, kc, ts(nch, 512)],
                                         start=(kc == 0), stop=(kc == 1))
                    nc.scalar.activation(out=pkT[:, mc, ts(nch, 512)], in_=ps,
                                         func=IDNT, bias=pkb_sb[:, mc])
            pv_sb = load2(pv_wT, D, "pv_sb")
            pvb_b = bc(pv_b[:, :], "pvb_b")
            pv_bp = bigp.tile([128, NTC, D], BF16, tag="bigC", name="pv_bp")
            for i in range(NTC):
                ps = ps4.tile([128, D], F32, tag="ps")
                for kc in range(2):
                    nc.tensor.matmul(ps, lhsT=aoT[:, kc, ts(i, 128)],
                                     rhs=pv_sb[:, kc], start=(kc == 0), stop=(kc == 1))
                nc.vector.tensor_add(pv_bp[:, i, :], ps, pvb_b)
            wps = t_p.tile([S, BPC], F32, tag="tp")
            for b in range(BPC):
                for kc in range(2):
                    nc.tensor.matmul(wps[:, b:b + 1],
                                     lhsT=pkT[:, kc, b * S:(b + 1) * S],
                                     rhs=pqT[:, kc, b:b + 1],
                                     start=(kc == 0), stop=(kc == 1))
            ewps = sb.tile([128, BPC], BF16, tag="ewps")
            nc.scalar.activation(out=ewps[0:S], in_=wps, func=EXP, scale=ISD)
            nc.sync.dma_start(out=ewps[S:2 * S], in_=ewps[0:S])
            ptrT_ps = t_p.tile([128, 96], F32, tag="p0", bufs=1)
            for b in range(BPC):
                i, r0 = b // 2, (b % 2) * 64
                lh = pv_bp[r0:r0 + 64, i]
                w1 = ewps[r0:r0 + 64, b:b + 1]
                nc.tensor.matmul(ptrT_ps[:, b:b + 1], lhsT=lh[:, 0:128], rhs=w1,
                                 start=True, stop=True)
                nc.tensor.matmul(ptrT_ps[:, 32 + b:33 + b], lhsT=lh[:, 128:256],
                                 rhs=w1, start=True, stop=True)
                nc.tensor.matmul(ptrT_ps[0:1, 64 + b:65 + b],
                                 lhsT=ones_sb[r0:r0 + 64, :], rhs=w1,
                                 start=True, stop=True)
            psum_sb = sb.tile([1, BPC], F32, tag="psmsb")
            nc.scalar.activation(out=psum_sb, in_=ptrT_ps[0:1, 64:96], func=COPY)
            prT = t_p.tile([BPC, 1], F32, tag="tp")
            nc.tensor.transpose(prT, psum_sb, i32f[:1, :1])
            prec = sb.tile([BPC, 1], F32, tag="prec")
            nc.vector.reciprocal(out=prec, in_=prT)
            ptrT_sb = sb.tile([128, 2, BPC], F32, tag="ptrTsb")
            nc.scalar.activation(out=ptrT_sb[:, 0], in_=ptrT_ps[:, 0:32], func=COPY)
            nc.scalar.activation(out=ptrT_sb[:, 1], in_=ptrT_ps[:, 32:64], func=COPY)
            ptr = cst.tile([BPC, D], F32, tag="ptr")
            for c in range(2):
                tpx = t_p.tile([BPC, 128], F32, tag="tp")
                nc.tensor.transpose(tpx, ptrT_sb[:, c], identF)
                nc.vector.tensor_scalar_mul(ptr[:, c * 128:(c + 1) * 128], tpx, prec)

            # ---- final MLP ----
            finT = cst.tile([128, 6, BPC], BF16, tag="finT")
            for piece, src in enumerate((menh, ptr, cur)):
                for c in range(2):
                    tp = t_p.tile([128, BPC], F32, tag="tp")
                    nc.tensor.transpose(tp, src[:, c * 128:(c + 1) * 128], i32f[:, :])
                    nc.scalar.activation(out=finT[:, piece * 2 + c], in_=tp, func=COPY)
            op1_sb = load2(op1_wT, 2 * D, "op1_sb", BF16, 6)
            op1b_sb = load2(op1_b, 1, "op1b_sb", F32, 4)
            hid = cst.tile([BPC, 2 * D], F32, tag="hid")
            for mc in range(4):
                ps = t_p.tile([128, BPC], F32, tag="tp")
                for kc in range(6):
                    nc.tensor.matmul(ps, lhsT=op1_sb[:, kc, ts(mc, 128)],
                                     rhs=finT[:, kc], start=(kc == 0), stop=(kc == 5))
                hT = sb.tile([128, BPC], F32, tag="hT")
                nc.scalar.activation(out=hT, in_=ps, func=GELU, bias=op1b_sb[:, mc])
                tp2 = t_p.tile([BPC, 128], F32, tag="tp")
                nc.tensor.transpose(tp2, hT, identF)
                nc.scalar.activation(out=hid[:, ts(mc, 128)], in_=tp2, func=COPY)
            nc.sync.dma_start(out=hid_o[:, :], in_=hid)
    nc.finalize()
    return nc


# ---------------------------------------------------------------------------
# Head program
# ---------------------------------------------------------------------------

def build_head():
    nc = bacc_mod.Bacc(None, target_bir_lowering=False, debug=True)
    hid = nc.dram_tensor("hid", [B, 2 * D], F32, kind="ExternalInput")
    op2_wTs = nc.dram_tensor("op2_wTs", [2 * D, RPAD], F32R, kind="ExternalInput")
    op2_bs = nc.dram_tensor("op2_bs", [1, RPAD], F32, kind="ExternalInput")
    mem = nc.dram_tensor("mem", [SLOTS, D], F32, kind="ExternalInput")
    ohwT = nc.dram_tensor("ohwT", [B, SLOTS], F32R, kind="ExternalInput")
    decay = nc.dram_tensor("decay", [SLOTS, 1], F32, kind="ExternalInput")
    cur = nc.dram_tensor("cur", [B, D], F32R, kind="ExternalInput")
    gate = nc.dram_tensor("gate", [B, 1], F32, kind="ExternalInput")
    logit_o = nc.dram_tensor("logit_o", [B, RPAD], F32, kind="ExternalOutput")
    mem_o = nc.dram_tensor("mem_o", [SLOTS, D], F32, kind="ExternalOutput")

    with TileContext(nc) as tc:
        with (tc.tile_pool(name="cst", bufs=1) as cst,
              tc.tile_pool(name="sb", bufs=3) as sb,
              tc.tile_pool(name="psp", bufs=3, space="PSUM") as psp,
              tc.tile_pool(name="tpp", bufs=2, space="PSUM") as tpp):
            ident = cst.tile([128, 128], F32, tag="ident")
            make_identity(nc, ident)
            hid_sb = cst.tile([128, 2, 2 * D], F32, tag="hid_sb")
            for c in range(2):
                nc.sync.dma_start(out=hid_sb[:, c], in_=hid[ts(c, 128), :])
            hidT = cst.tile([128, 4, B], F32R, tag="hidT")
            for kc in range(4):
                for c in range(2):
                    tp = tpp.tile([128, 128], F32, tag="tp")
                    nc.tensor.transpose(tp, hid_sb[:, c, ts(kc, 128)], ident)
                    nc.scalar.activation(out=hidT[:, kc, ts(c, 128)], in_=tp, func=COPY)
            w_sb = cst.tile([128, 4, RPAD], F32R, tag="w_sb")
            for c in range(4):
                nc.sync.dma_start(out=w_sb[:, c], in_=op2_wTs[ts(c, 128), :])
            ob_sb = cst.tile([128, RPAD], F32, tag="ob_sb")
            nc.sync.dma_start(out=ob_sb, in_=_bcast_ap(op2_bs[:, :]))
            for mc in range(2):
                for nch in range(3):
                    nw = 512 if nch < 2 else 256
                    ncol = ds(nch * 512, nw)
                    ps = psp.tile([128, 512], F32, tag="ps")
                    for kc in range(4):
                        nc.tensor.matmul(ps[:, 0:nw], lhsT=hidT[:, kc, ts(mc, 128)],
                                         rhs=w_sb[:, kc, ncol],
                                         start=(kc == 0), stop=(kc == 3))
                    lo = sb.tile([128, 512], F32, tag="lo")
                    nc.vector.tensor_add(lo[:, 0:nw], ps[:, 0:nw], ob_sb[:, ncol])
                    nc.sync.dma_start(out=logit_o[ts(mc, 128), ncol], in_=lo[:, 0:nw])
            # EMA
            cur_sb = cst.tile([128, 2, D], F32R, tag="cur_sb")
            gate_sb = cst.tile([128, 2, 1], F32, tag="gate_sb")
            for c in range(2):
                nc.sync.dma_start(out=cur_sb[:, c], in_=cur[ts(c, 128), :])
                nc.sync.dma_start(out=gate_sb[:, c], in_=gate[ts(c, 128), :])
            for c in range(2):
                nc.vector.tensor_scalar_mul(cur_sb[:, c], cur_sb[:, c], gate_sb[:, c])
            ohwT_sb = cst.tile([128, 2, SLOTS], F32R, tag="ohwT_sb")
            for c in range(2):
                nc.sync.dma_start(out=ohwT_sb[:, c], in_=ohwT[ts(c, 128), :])
            mem_sb = cst.tile([128, 4, D], F32, tag="mem_sb")
            dec_sb = cst.tile([128, 4, 1], F32, tag="dec_sb")
            for c in range(4):
                nc.sync.dma_start(out=mem_sb[:, c], in_=mem[ts(c, 128), :])
                nc.sync.dma_start(out=dec_sb[:, c], in_=decay[ts(c, 128), :])
            for sc in range(4):
                ps = psp.tile([128, D], F32, tag="ps")
                for kc in range(2):
                    nc.tensor.matmul(ps, lhsT=ohwT_sb[:, kc, ts(sc, 128)],
                                     rhs=cur_sb[:, kc], start=(kc == 0), stop=(kc == 1))
                nc.vector.tensor_scalar_mul(mem_sb[:, sc], mem_sb[:, sc], dec_sb[:, sc])
                nc.vector.tensor_add(mem_sb[:, sc], mem_sb[:, sc], ps)
                nc.sync.dma_start(out=mem_o[ts(sc, 128), :], in_=mem_sb[:, sc])
    nc.finalize()
    return nc


# ---------------------------------------------------------------------------
# Host orchestration
# ---------------------------------------------------------------------------

_CACHE = {}
EXEC_TIMES = []


def _get(name, builder, *a):
    if name not in _CACHE:
        _CACHE[name] = builder(*a)
    return _CACHE[name]


def _run(nc, in_maps):
    import time as _time
    t0 = _time.perf_counter()
    r = run_bass_kernel_spmd(nc, in_maps, CORE_IDS)
    EXEC_TIMES.append(int((_time.perf_counter() - t0) * 1e9))
    return r.results


def _pad_rows(a, n):
    out = np.zeros((n, a.shape[1]), np.float32)
    out[:a.shape[0]] = np.asarray(a, np.float32)
    return out


def kernel(locations, users, start_mins, weekdays, target, base_embed, adj_left,
           adj_right, gc1_w, gc1_b, gc2_w, gc2_b, user_emb, hour_emb, wd_emb,
           memory, rq_w, rq_b, rk_w, rk_b, wg_w, wg_b, pq_w, pq_b, pk_w, pk_b,
           pv_w, pv_b, wih0, whh0, bih0, bhh0, wih1, whh1, bih1, bhh1, ain_w,
           ain_b, aout_w, aout_b, n1g, n1b, n2g, n2b, n3g, n3b, op1_w, op1_b,
           op2_w, op2_b):
    f32 = np.float32
    A = lambda v: np.ascontiguousarray(v, f32)
    row = lambda v: A(np.reshape(v, (1, -1)))
    col = lambda v: A(np.reshape(v, (-1, 1)))
    adj_leftT = A(np.asarray(adj_left).T)

    hop1 = _get("hop1", build_hop, True)
    hop2 = _get("hop2", build_hop, False)
    in1 = []
    for c in range(NC):
        lb = np.zeros((RANK, RPAD), f32)
        lb[:, :RPC] = adj_leftT[:, c * RPC:(c + 1) * RPC]
        in1.append(dict(lbT=lb, radj=A(adj_right), xin=_pad_rows(np.asarray(base_embed), KPAD).astype(ml_dtypes.bfloat16),
                        gw=A(np.asarray(gc1_w).T), gb=row(gc1_b)))
    r1 = _run(hop1, in1)
    nb1 = np.concatenate([r1[c]["out"][:RPC] for c in range(NC)], 0)

    in2 = [dict(lbT=in1[c]["lbT"], radj=in1[c]["radj"], xin=_pad_rows(nb1, KCH * 128).astype(ml_dtypes.bfloat16),
                gw=A(np.asarray(gc2_w).T), gb=row(gc2_b),
                recip_i=r1[c]["recip_o"],
                basei=_pad_rows(np.asarray(base_embed)[c * RPC:(c + 1) * RPC], RPAD))
           for c in range(NC)]
    r2 = _run(hop2, in2)
    Y = np.concatenate([r2[c]["out"][:RPC] for c in range(NC)], 0)

    hours = (np.asarray(start_mins) // 60) % 24
    x = (Y[np.asarray(locations)]
         + np.asarray(user_emb)[np.asarray(users)[:, 0]][:, None, :]
         + np.asarray(hour_emb)[hours]
         + np.asarray(wd_emb)[np.asarray(weekdays)]).astype(f32)

    affine = not all(np.allclose(g, 1) and np.allclose(bb, 0)
                     for g, bb in ((n1g, n1b), (n2g, n2b), (n3g, n3b)))
    seq = _get(f"seq{affine}", build_seq, affine)
    bf = ml_dtypes.bfloat16
    common = dict(
        wihT0=np.ascontiguousarray(np.asarray(wih0, f32).T).astype(bf),
        whhT0=np.ascontiguousarray(np.asarray(whh0, f32).T).astype(bf),
        wihT1=np.ascontiguousarray(np.asarray(wih1, f32).T).astype(bf),
        whhT1=np.ascontiguousarray(np.asarray(whh1, f32).T).astype(bf),
        bias0=row(np.asarray(bih0) + np.asarray(bhh0)),
        bias1=row(np.asarray(bih1) + np.asarray(bhh1)),
        ain_wT=A(np.asarray(ain_w).T[:, 0:2 * D]),
        av_wT=A(np.asarray(ain_w).T[:, 2 * D:3 * D]),
        ain_qkb=col(np.asarray(ain_b)[0:2 * D]), ain_vb=row(np.asarray(ain_b)[2 * D:]),
        aout_wT=A(np.asarray(aout_w).T), aout_b=row(aout_b),
        n1g=row(n1g), n1b=row(n1b), n2g=row(n2g), n2b=row(n2b),
        n3g=row(n3g), n3b=row(n3b),
        rq_wT=A(np.asarray(rq_w).T), rq_b=col(rq_b),
        rk_wT=A(np.asarray(rk_w).T), rk_b=col(rk_b),
        memT=A(np.asarray(memory).T),
        mem_aug=A(np.concatenate([np.asarray(memory, f32),
                                  np.ones((SLOTS, 1), f32),
                                  np.zeros((SLOTS, 1), f32)], 1)),
        wg_wT=A(np.asarray(wg_w).T), wg_b=A(np.asarray(wg_b)).reshape(1, 1),
        pq_wT=A(np.asarray(pq_w).T), pq_b=col(pq_b),
        pk_wT=A(np.asarray(pk_w).T), pk_b=col(pk_b),
        pv_wT=A(np.asarray(pv_w).T), pv_b=row(pv_b),
        op1_wT=np.ascontiguousarray(np.asarray(op1_w, f32).T).astype(bf),
        op1_b=col(op1_b),
        onesc=np.ones((128, 1), f32).astype(bf),
    )
    in3 = []
    for c in range(NC):
        xb = x[c * BPC:(c + 1) * BPC]
        in3.append(dict(common, x_bp=A(xb.reshape(NT, D)),
                        xT=np.ascontiguousarray(
                            xb.transpose(2, 1, 0).reshape(D, 4, 16, BPC)
                            .transpose(0, 2, 1, 3).reshape(D, S, BPC)).astype(bf)))
    r3 = _run(seq, in3)
    hid = np.concatenate([r3[c]["hid_o"] for c in range(NC)], 0)
    cur = np.concatenate([r3[c]["cur_o"] for c in range(NC)], 0)
    gate = np.concatenate([r3[c]["gate_o"][0] for c in range(NC)], 0)

    addr = (np.asarray(target) % SLOTS).astype(np.int64)
    later = np.zeros(B, f32)
    count = np.zeros(SLOTS, np.int64)
    for b in range(B - 1, -1, -1):
        later[b] = count[addr[b]]
        count[addr[b]] += 1
    coeff = (0.1 * np.power(0.9, later)).astype(f32)
    decay = np.power(0.9, count.astype(f32)).astype(f32)
    ohwT = np.zeros((B, SLOTS), f32)
    ohwT[np.arange(B), addr] = coeff

    head = _get("head", build_head)
    op2_wT = A(np.asarray(op2_w).T)
    in4 = []
    for c in range(NC):
        ws = np.zeros((2 * D, RPAD), f32)
        ws[:, :RPC] = op2_wT[:, c * RPC:(c + 1) * RPC]
        bs = np.zeros((1, RPAD), f32)
        bs[0, :RPC] = np.asarray(op2_b, f32)[c * RPC:(c + 1) * RPC]
        in4.append(dict(hid=hid, op2_wTs=ws, op2_bs=bs, mem=A(memory), ohwT=ohwT,
                        decay=col(decay), cur=cur, gate=col(gate)))
    r4 = _run(head, in4)
    logits = np.concatenate([r4[c]["logit_o"][:, :RPC] for c in range(NC)], 1)
    new_mem = r4[0]["mem_o"]
    return logits, new_mem
